# revision 66
# baseline (speedup 1.0000x reference)
"""Multi-Head Latent Attention on 8 Trainium2 NeuronCores.

Sharding: core c = (batch b = c//4) x (head-group g = c%4, 4 heads each).

The kv down projection is token-sharded across the 4 cores of a batch
group: core (b, r) computes kv_c^T for token slices {512j + 128r : j} and
the full kv_c is assembled with four pipelined AllGathers (one per
512-token chunk, 0.5 MB each), overlapped with compute.  The query path
needs no collective at all: the host folds W_q = query_down_w @
query_up_w (and the rope analog) so each core computes its 4 heads'
queries directly from x^T — per-core FLOPs for the query side are lower
than sharing the 1536-dim latent, since only 768 output dims are needed.
Each core computes K/V/rope/attention for its 4 heads and a partial
output projection; the host sums the 4 bf16 partials per batch and adds
the output bias (plus the value-up bias folded through out_w, exact
because softmax rows sum to 1).

Phases are kept temporally separate (projections | attention | out-proj)
because the clock governor duty-cycles the PE when many engines run
concurrently; pure-matmul stretches run at a higher p-state.

All layouts are feature-major: x^T, kv_c^T, K^T, Q^T, ctx^T, out^T.
Scores are computed as scores^T[k, q] so probs^T feeds the context
matmul directly.  Softmax denominators are accumulated on the vector
engine (bf16) and reduced across partitions with one ones-matmul per
(chunk, head).  Exp is applied without max-subtraction (scores for this
problem are in [-1, 1]).

Rope is applied token-major: projections computed once with tokens on
partitions (weights streaming), rotated pairwise on the vector engine,
transposed back to feature-major with PE transposes (deferred so the PE
never waits on the rotation chain).
"""

import numpy as np
import ml_dtypes

import concourse.bass as bass
import concourse.mybir as mybir
from concourse.tile import TileContext
from concourse.bass_utils import run_bass_kernel_spmd

F32 = mybir.dt.float32
BF16 = mybir.dt.bfloat16
AF = mybir.ActivationFunctionType
BF = ml_dtypes.bfloat16

HIDDEN = 2048
NUM_HEADS = 16
HEAD_DIM = 128
KV_C = 512
Q_C = 1536
ROPE_DIM = 64
B, S = 2, 2048

P = 128
NH = 4          # heads per core
SC = 512        # token chunk (one AllGather / one attention q-chunk)
NKT = HIDDEN // P       # 16 k-tiles over x features
NKV = KV_C // P         # 4 kv_c feature tiles
NSC = S // SC           # 4 chunks
NT = S // P             # 16 token tiles
SCALE = float(1.0 / np.sqrt(HEAD_DIM + ROPE_DIM))
NEG = -1.0e5
GROUPS = [[0, 1, 2, 3], [4, 5, 6, 7]]


def _split_waits(nc, maxw=1):
    """This container's walrus accepts at most one sem-wait per instruction;
    move excess waits onto same-engine NOPs inserted immediately before."""
    for fn in nc.m.functions:
        for bb in fn.blocks:
            newlist = []
            for ins in bb.instructions:
                si = ins.sync_info
                if si is not None and si.on_wait is not None and len(si.on_wait) > maxw:
                    waits = list(si.on_wait)
                    extra, keep = waits[:-maxw], waits[-maxw:]
                    for k, i in enumerate(range(0, len(extra), maxw)):
                        nop = mybir.InstNoOp(
                            name=f"{ins.name}-waitsplit-{k}", ins=[], outs=[]
                        )
                        nop.engine = ins.engine
                        nop.sync_info = mybir.SyncInfo(
                            on_wait=extra[i : i + maxw], on_update=[]
                        )
                        newlist.append(nop)
                    ins.sync_info = mybir.SyncInfo(
                        on_wait=keep, on_update=list(si.on_update or [])
                    )
                newlist.append(ins)
            bb.instructions = newlist


def build():
    nc = bass.Bass(num_devices=8)
    dt = nc.dram_tensor
    xT = dt("xT", [P, NSC, NKT, SC], BF16, kind="ExternalInput")
    xTo = dt("xTo", [P, NSC, NKT, P], BF16, kind="ExternalInput")
    Wdkv = dt("Wdkv", [P, NKV, NKT, P], BF16, kind="ExternalInput")
    bdkv = dt("bdkv", [P, NKV], F32, kind="ExternalInput")
    Wku = dt("Wku", [P, NKV, NH * HEAD_DIM], BF16, kind="ExternalInput")
    bku = dt("bku", [P, 4], F32, kind="ExternalInput")
    Wvu = dt("Wvu", [P, NKV, NH * HEAD_DIM], BF16, kind="ExternalInput")
    Wkr = dt("Wkr", [P, NKV, NH * ROPE_DIM], BF16, kind="ExternalInput")
    Wq = dt("Wq", [P, 2, NKT, 256], BF16, kind="ExternalInput")
    bq = dt("bq", [P, 4], F32, kind="ExternalInput")
    Wqr = dt("Wqr", [P, NKT, NH * ROPE_DIM], BF16, kind="ExternalInput")
    # rope biases, token-major broadcast: [P, NH*ROPE_DIM] (same per token)
    bkrT = dt("bkrT", [P, NH * ROPE_DIM], F32, kind="ExternalInput")
    bqrT = dt("bqrT", [P, NH * ROPE_DIM], F32, kind="ExternalInput")
    Wo = dt("Wo", [P, NKT, NH, P], BF16, kind="ExternalInput")
    # token-major cos/sin: [P tokens-within-tile, tile, NH, ROPE_DIM/2]
    cosT = dt("cosT", [P, NT * NH * (ROPE_DIM // 2)], BF16, kind="ExternalInput")
    sinT = dt("sinT", [P, NT * NH * (ROPE_DIM // 2)], BF16, kind="ExternalInput")
    tri = dt("tri", [P, P], F32, kind="ExternalInput")
    eye = dt("eye", [P, P], BF16, kind="ExternalInput")
    outT = dt("outT", [HIDDEN, S], BF16, kind="ExternalOutput")

    cc_in = [dt(f"cc_in{j}", [P, NKV, P], BF16, kind="Internal")
             for j in range(NSC)]
    cc_out = [dt(f"cc_out{j}", [4, P, NKV, P], BF16, kind="Internal")
              for j in range(NSC)]

    with TileContext(nc) as tc:
        with (
            tc.tile_pool(name="const", bufs=1) as pc,
            tc.tile_pool(name="qkv", bufs=1) as pq,
        ):
            # --- mid-lived scope: phase 1 + projections (loop 1) ----------
            pmid = tc.alloc_tile_pool(name="pmid", bufs=1)

            # kv-down weights + bias first on gpsimd (phase-1 critical)
            bdkv_sb = pc.tile([P, NKV], F32)
            nc.gpsimd.dma_start(bdkv_sb[:], bdkv[:])
            wdkv_t = pmid.tile([P, NKT, KV_C], BF16, tag="wdkv")
            for q4 in range(4):
                nc.gpsimd.dma_start(
                    wdkv_t[:, :, q4 * P : (q4 + 1) * P], Wdkv[:, q4]
                )
            # remaining constants (gpsimd queue, needed later)
            bq_sb = pc.tile([P, 4], F32)
            nc.gpsimd.dma_start(bq_sb[:], bq[:])
            bku_sb = pc.tile([P, 4], F32)
            nc.gpsimd.dma_start(bku_sb[:], bku[:])
            tri_sb = pc.tile([P, P], F32)
            nc.gpsimd.dma_start(tri_sb[:], tri[:])
            eye_sb = pc.tile([P, P], BF16)
            nc.gpsimd.dma_start(eye_sb[:], eye[:])
            ones_sb = pc.tile([P, P], BF16)
            nc.vector.memset(ones_sb[:], 1.0)
            bkrT_sb = pc.tile([P, NH * ROPE_DIM], F32)
            nc.gpsimd.dma_start(bkrT_sb[:], bkrT[:])
            bqrT_sb = pc.tile([P, NH * ROPE_DIM], F32)
            nc.gpsimd.dma_start(bqrT_sb[:], bqrT[:])

            # persistent across attention: K/V and per-chunk Q/ctx tiles
            kc_sb = pq.tile([P, NH, S], BF16)
            kr_sb = pq.tile([P, 2, S], BF16)
            v_sb = pq.tile([P, NT, NH * HEAD_DIM], BF16)

            cos_sb = pmid.tile([P, NT, NH, ROPE_DIM // 2], BF16, tag="cos")
            nc.gpsimd.dma_start(
                cos_sb[:], cosT.rearrange("p (t h i) -> p t h i", t=NT, h=NH)
            )
            sin_sb = pmid.tile([P, NT, NH, ROPE_DIM // 2], BF16, tag="sin")
            nc.gpsimd.dma_start(
                sin_sb[:], sinT.rearrange("p (t h i) -> p t h i", t=NT, h=NH)
            )
            kv_sb = pmid.tile([P, NKV, S], BF16, tag="kv")

            def load_xt_chunk(qc):
                t = pmid.tile([P, NKT, SC], BF16, tag="xt", bufs=2)
                nc.scalar.dma_start(t[:], xT[:, qc])
                return t

            # remaining weights, spread across the three DMA-capable queues
            # so the phase-1/loop-1 critical loads land early
            wq_t = pmid.tile([P, NKT, NH * HEAD_DIM], BF16, tag="wq")
            wqr_t = pmid.tile([P, NKT, NH * ROPE_DIM], BF16, tag="wqr")
            nc.gpsimd.dma_start(wqr_t[:], Wqr[:])
            wku_t = pmid.tile([P, NKV, NH * HEAD_DIM], BF16, tag="wku")
            wvu_t = pmid.tile([P, NKV, NH * HEAD_DIM], BF16, tag="wvu")
            wkr_t = pmid.tile([P, NKV, NH * ROPE_DIM], BF16, tag="wkr")

            # ---------------- phase 1: kv down projection + AllGather -----
            with (
                tc.tile_pool(name="p1", bufs=1) as p1,
                tc.tile_pool(name="ps1", bufs=6, space="PSUM") as ps1,
            ):
                # own token columns for the kv down projection (host-packed),
                # split across sync+scalar queues to land fast
                xto = p1.tile([P, NKT, NSC * P], BF16)
                for j in range(NSC):
                    eng = nc.sync if j % 2 == 0 else nc.scalar
                    eng.dma_start(
                        xto[:, :, j * P : (j + 1) * P], xTo[:, j]
                    )
                # first x chunk + query weights on scalar, behind the
                # critical xto halves; sync stays clear for cc_in posts
                xt0 = load_xt_chunk(0)
                for h2 in range(2):
                    nc.scalar.dma_start(
                        wq_t[:, :, h2 * 256 : (h2 + 1) * 256], Wq[:, h2]
                    )
                nc.scalar.dma_start(wku_t[:], Wku[:])
                nc.scalar.dma_start(wvu_t[:], Wvu[:])
                nc.scalar.dma_start(wkr_t[:], Wkr[:])
                kv_own = p1.tile([P, NKV, NSC * P], BF16)
                for j in range(NSC):
                    for m in range(NKV):
                        ps = ps1.tile([P, P], F32, tag="mm")
                        for k in range(NKT):
                            nc.tensor.matmul(
                                ps[:],
                                wdkv_t[:, k, m * P : (m + 1) * P],
                                xto[:, k, j * P : (j + 1) * P],
                                start=(k == 0),
                                stop=(k == NKT - 1),
                            )
                        nc.vector.tensor_scalar_add(
                            kv_own[:, m, j * P : (j + 1) * P],
                            ps[:],
                            bdkv_sb[:, m : m + 1],
                        )
                    nc.sync.dma_start(
                        cc_in[j][:], kv_own[:, :, j * P : (j + 1) * P]
                    )
                for j in range(NSC):
                    nc.gpsimd.collective_compute(
                        "AllGather",
                        mybir.AluOpType.bypass,
                        replica_groups=GROUPS,
                        ins=[cc_in[j][:].opt()],
                        outs=[cc_out[j][:].opt()],
                    )
                for j in range(NSC):
                    for rr in range(4):
                        c0 = j * SC + rr * P
                        nc.sync.dma_start(
                            kv_sb[:, :, c0 : c0 + P], cc_out[j][rr]
                        )

            # --------- loop 1: all projections (PE-dense) -----------------
            q_tiles = []
            with (
                tc.tile_pool(name="p2t", bufs=3) as p2t,
                tc.tile_pool(name="psP", bufs=1, space="PSUM") as psP,
            ):

                def rope_project(tt, nk, src, src_t, w_t, bias_sb):
                    """Project rope dims token-major for global tile tt and
                    rotate on DVE; returns the rotated bf16 tile."""
                    ps = psP.tile([P, NH * ROPE_DIM], F32, tag="mm", bufs=4)
                    for k in range(nk):
                        nc.tensor.matmul(
                            ps[:],
                            src[:, k, src_t * P : (src_t + 1) * P],
                            w_t[:, k, :],
                            start=(k == 0),
                            stop=(k == nk - 1),
                        )
                    pre = p2t.tile([P, NH * ROPE_DIM], BF16, tag="ropeadd")
                    nc.vector.tensor_tensor(
                        pre[:], ps[:], bias_sb[:], mybir.AluOpType.add
                    )
                    prr = pre.rearrange("p (h i two) -> p h i two", h=NH, two=2)
                    rot = p2t.tile([P, NH, ROPE_DIM // 2, 2], BF16, tag="rot",
                                   bufs=5)
                    t1 = p2t.tile([P, NH, ROPE_DIM // 2], BF16, tag="ropet1")
                    nc.vector.tensor_tensor(
                        t1[:], prr[:, :, :, 0], cos_sb[:, tt],
                        mybir.AluOpType.mult,
                    )
                    t2 = p2t.tile([P, NH, ROPE_DIM // 2], BF16, tag="ropet2")
                    nc.vector.tensor_tensor(
                        t2[:], prr[:, :, :, 1], sin_sb[:, tt],
                        mybir.AluOpType.mult,
                    )
                    nc.vector.tensor_tensor(
                        rot[:, :, :, 0], t1[:], t2[:], mybir.AluOpType.subtract
                    )
                    nc.vector.tensor_tensor(
                        t1[:], prr[:, :, :, 0], sin_sb[:, tt],
                        mybir.AluOpType.mult,
                    )
                    nc.vector.tensor_tensor(
                        t2[:], prr[:, :, :, 1], cos_sb[:, tt],
                        mybir.AluOpType.mult,
                    )
                    nc.vector.tensor_tensor(
                        rot[:, :, :, 1], t1[:], t2[:], mybir.AluOpType.add
                    )
                    return rot

                def rope_store(rot, dst_sb, dst_t):
                    """Transpose the rotated tile back to feature-major."""
                    rotf = rot.rearrange("p h i two -> p (h i two)")
                    for hb in range(2):
                        pt = psP.tile([P, P], BF16, tag="tp", bufs=4)
                        nc.tensor.transpose(
                            pt[:], rotf[:, hb * P : (hb + 1) * P], eye_sb[:]
                        )
                        nc.scalar.copy(
                            dst_sb[:, hb, dst_t * P : (dst_t + 1) * P], pt[:]
                        )

                xt_cur = xt0
                for qc in range(NSC):
                    qsl = slice(qc * SC, (qc + 1) * SC)
                    xt = xt_cur
                    if qc + 1 < NSC:
                        xt_cur = load_xt_chunk(qc + 1)
                    # queries for this chunk: direct from x, no AG dep
                    qc_sb = pq.tile([P, NH, SC], BF16, tag="qc", bufs=4)
                    qr_sb = pq.tile([P, 2, SC], BF16, tag="qr", bufs=4)
                    q_tiles.append((qc_sb, qr_sb))
                    for m in range(NH):
                        ps = psP.tile([P, SC], F32, tag="mm", bufs=4)
                        for k in range(NKT):
                            nc.tensor.matmul(
                                ps[:],
                                wq_t[:, k, m * P : (m + 1) * P],
                                xt[:, k, :],
                                start=(k == 0),
                                stop=(k == NKT - 1),
                            )
                        nc.vector.tensor_scalar_add(
                            qc_sb[:, m, :], ps[:], bq_sb[:, m : m + 1]
                        )
                    rots = [
                        rope_project(qc * 4 + t, NKT, xt, t, wqr_t, bqrT_sb)
                        for t in range(4)
                    ]
                    for t in range(4):
                        rope_store(rots[t], qr_sb, t)

                    # keys/values for this chunk (needs AllGather qc)
                    for m in range(NH):
                        ps = psP.tile([P, SC], F32, tag="mm", bufs=4)
                        for k in range(NKV):
                            nc.tensor.matmul(
                                ps[:],
                                wku_t[:, k, m * P : (m + 1) * P],
                                kv_sb[:, k, qsl],
                                start=(k == 0),
                                stop=(k == NKV - 1),
                            )
                        nc.vector.tensor_scalar_add(
                            kc_sb[:, m, qsl], ps[:], bku_sb[:, m : m + 1]
                        )
                    for t in range(4):
                        tt = qc * 4 + t
                        ps = psP.tile([P, NH * HEAD_DIM], F32, tag="mm",
                                      bufs=4)
                        for k in range(NKV):
                            nc.tensor.matmul(
                                ps[:],
                                kv_sb[:, k, tt * P : (tt + 1) * P],
                                wvu_t[:, k, :],
                                start=(k == 0),
                                stop=(k == NKV - 1),
                            )
                        nc.vector.tensor_copy(v_sb[:, tt, :], ps[:])
                    rots = [
                        rope_project(qc * 4 + t, NKV, kv_sb, qc * 4 + t,
                                     wkr_t, bkrT_sb)
                        for t in range(4)
                    ]
                    for t in range(4):
                        rope_store(rots[t], kr_sb, qc * 4 + t)
            pmid.release()

            # --------- loop 2: attention --------------------------------
            ctx_tiles = []
            patx = tc.alloc_tile_pool(name="patx", bufs=1)
            with (
                tc.tile_pool(name="pat", bufs=8) as pat,
                tc.tile_pool(name="patt", bufs=2) as patt,
                tc.tile_pool(name="psL2", bufs=1, space="PSUM") as psL2,
            ):
                for qc in range(NSC):
                    qc_sb, qr_sb = q_tiles[qc]
                    nkb = 4 * qc + 4
                    ctx_q = patx.tile([P, NH, SC], BF16, tag="ctx", bufs=4)
                    ctx_tiles.append(ctx_q)
                    for h in range(NH):
                        hc = h // 2
                        hp = (h % 2) * ROPE_DIM
                        psum_ctx = psL2.tile([P, SC], F32, tag="ctx", bufs=2)
                        acc = patt.tile([P, SC], BF16, tag="acc")

                        def score_mm(dst, kb, c):
                            nc.tensor.matmul(
                                dst[:, c:],
                                kc_sb[:, h, kb * P : (kb + 1) * P],
                                qc_sb[:, h, c:],
                                start=True, stop=False,
                            )
                            nc.tensor.matmul(
                                dst[:, c:],
                                kr_sb[hp : hp + ROPE_DIM, hc,
                                      kb * P : (kb + 1) * P],
                                qr_sb[hp : hp + ROPE_DIM, hc, c:],
                                start=False, stop=True,
                            )

                        def acc_ctx(probs, kb, c):
                            if kb == 0:
                                nc.vector.tensor_copy(acc[:], probs[:])
                            else:
                                nc.vector.tensor_tensor(
                                    acc[:, c:], acc[:, c:], probs[:, c:],
                                    mybir.AluOpType.add,
                                )
                            nc.tensor.matmul(
                                psum_ctx[:, c:],
                                v_sb[:, kb, h * P : (h + 1) * P],
                                probs[:, c:],
                                start=(kb == 0), stop=(kb == nkb - 1),
                            )

                        # Emit score-matmuls one step ahead of the ctx
                        # matmuls so the PE never waits on the exp latency.
                        nfull = 4 * qc
                        steps = [("pair", kb) for kb in range(0, nfull, 2)]
                        steps += [("diag", kb) for kb in range(nfull, nkb)]

                        def emit_scores(step):
                            kind, kb = step
                            if kind == "pair":
                                ps = psL2.tile([P, 2, SC], F32, tag="sc2",
                                               bufs=2)
                                score_mm(ps[:, 0], kb, 0)
                                score_mm(ps[:, 1], kb + 1, 0)
                                probs2 = pat.tile([P, 2, SC], BF16,
                                                  tag="probs2", bufs=4)
                                nc.scalar.activation(
                                    probs2[:], ps[:], AF.Exp, scale=SCALE
                                )
                                return probs2
                            ps = psL2.tile([P, SC], F32, tag="sc", bufs=2)
                            c = (kb - 4 * qc) * P
                            score_mm(ps, kb, c)
                            nc.vector.tensor_tensor(
                                ps[:, c : c + P],
                                ps[:, c : c + P],
                                tri_sb[:],
                                mybir.AluOpType.add,
                            )
                            probs = pat.tile([P, SC], BF16, tag="probs",
                                             bufs=4)
                            nc.scalar.activation(
                                probs[:, c:], ps[:, c:], AF.Exp, scale=SCALE
                            )
                            return probs

                        def emit_ctx(step, probs):
                            kind, kb = step
                            if kind == "pair":
                                acc_ctx(probs[:, 0], kb, 0)
                                acc_ctx(probs[:, 1], kb + 1, 0)
                            else:
                                acc_ctx(probs, kb, (kb - 4 * qc) * P)

                        prev = None
                        for step in steps:
                            probs = emit_scores(step)
                            if prev is not None:
                                emit_ctx(*prev)
                            prev = (step, probs)
                        emit_ctx(*prev)
                        psd = psL2.tile([P, SC], F32, tag="sc", bufs=2)
                        nc.tensor.matmul(
                            psd[:], ones_sb[:], acc[:], start=True, stop=True
                        )
                        rcp = patt.tile([P, SC], F32, tag="rcp", bufs=2)
                        nc.vector.reciprocal(rcp[:], psd[:])
                        nc.vector.tensor_tensor(
                            ctx_q[:, h, :], psum_ctx[:], rcp[:],
                            mybir.AluOpType.mult,
                        )

            # --------- loop 3: out-projection (pure matmul) ---------------
            with (
                tc.tile_pool(name="pout", bufs=6) as pout,
                tc.tile_pool(name="pow", bufs=3) as pow_,
                tc.tile_pool(name="psL3", bufs=1, space="PSUM") as psL3,
            ):
                for m in range(NKT):
                    wo_t = pow_.tile([P, NH, P], BF16, tag="wo")
                    nc.gpsimd.dma_start(wo_t[:], Wo[:, m])
                    for qc in range(NSC):
                        ps = psL3.tile([P, SC], F32, tag="m", bufs=4)
                        for k in range(NH):
                            nc.tensor.matmul(
                                ps[:],
                                wo_t[:, k, :],
                                ctx_tiles[qc][:, k, :],
                                start=(k == 0),
                                stop=(k == NH - 1),
                            )
                        og = pout.tile([P, SC], BF16, tag="og")
                        nc.vector.tensor_copy(og[:], ps[:])
                        nc.sync.dma_start(
                            outT[m * P : (m + 1) * P,
                                 qc * SC : (qc + 1) * SC],
                            og[:],
                        )
            patx.release()
    _split_waits(nc)
    return nc


def _col_bias(b, nm):
    """[nm*128] -> [128, nm] (column m = bias for feature chunk m)."""
    return np.ascontiguousarray(b.reshape(nm, P).T).astype(np.float32)


def _pack(w, msplit=None):
    """[K, M] -> [P, (M//msplit), K//P, msplit] if msplit else [P, K//P, M]:
    partition-major device layout so DMAs are contiguous per partition."""
    K, M = w.shape
    a = w.reshape(K // P, P, M)
    if msplit is None:
        return np.ascontiguousarray(a.transpose(1, 0, 2)).astype(BF)
    a = a.reshape(K // P, P, M // msplit, msplit)
    return np.ascontiguousarray(a.transpose(1, 2, 0, 3)).astype(BF)


_NC = None


def kernel(**inputs):
    global _NC
    inp = {k: np.asarray(v) for k, v in inputs.items()}
    x = inp["x"].astype(np.float32)

    # token-major cos/sin: cosT[p, (t, h, i)] = cos(pos(t*128+p)*inv_freq[i])
    pos = np.arange(S, dtype=np.float64)
    inv = 1.0 / (10000.0 ** (np.arange(0, ROPE_DIM, 2, np.float64) / ROPE_DIM))
    ang = pos[:, None] * inv[None, :]              # [S, 32]
    cosS = np.cos(ang).reshape(NT, P, ROPE_DIM // 2)  # [t, p, i]
    sinS = np.sin(ang).reshape(NT, P, ROPE_DIM // 2)
    cosT = np.repeat(
        cosS.transpose(1, 0, 2)[:, :, None, :], NH, axis=2
    ).reshape(P, -1).astype(BF)
    sinT = np.repeat(
        sinS.transpose(1, 0, 2)[:, :, None, :], NH, axis=2
    ).reshape(P, -1).astype(BF)
    tri = np.where(
        np.arange(P)[None, :] >= np.arange(P)[:, None], 0.0, NEG
    ).astype(np.float32)
    eye = np.eye(P, dtype=np.float32).astype(BF)

    qdw = inp["query_down_w"].astype(np.float32)
    qdb = inp["query_down_b"].astype(np.float32)

    in_maps = []
    for c in range(8):
        b, r = c // 4, c % 4
        h0 = r * NH
        csl = slice(h0 * HEAD_DIM, (h0 + NH) * HEAD_DIM)
        rsl = slice(h0 * ROPE_DIM, (h0 + NH) * ROPE_DIM)
        # fold the query path: Q = x @ (qdw @ qu) + (qdb @ qu + qub)
        wq = qdw @ inp["query_up_w"][:, csl].astype(np.float32)
        bq_f = qdb @ inp["query_up_w"][:, csl].astype(np.float32) \
            + inp["query_up_b"][csl].astype(np.float32)
        wqr = qdw @ inp["query_rope_w"][:, rsl].astype(np.float32)
        bqr_f = qdb @ inp["query_rope_w"][:, rsl].astype(np.float32) \
            + inp["query_rope_b"][rsl].astype(np.float32)
        own_cols = np.concatenate(
            [np.arange(SC * j + P * r, SC * j + P * r + P) for j in range(NSC)]
        )
        in_maps.append(
            {
                "xT": _pack(x[b].T, SC),
                "xTo": _pack(x[b].T[:, own_cols], P),
                "Wdkv": _pack(inp["kv_down_w"], P),
                "bdkv": _col_bias(inp["kv_down_b"], NKV),
                "Wku": _pack(inp["key_up_w"][:, csl]),
                "bku": _col_bias(inp["key_up_b"][csl], 4),
                "Wvu": _pack(inp["value_up_w"][:, csl]),
                "Wkr": _pack(inp["key_rope_w"][:, rsl]),
                "Wq": _pack(wq, 256),
                "bq": _col_bias(bq_f, 4),
                "Wqr": _pack(wqr),
                "bkrT": np.broadcast_to(
                    inp["key_rope_b"][rsl].astype(np.float32),
                    (P, NH * ROPE_DIM),
                ).copy(),
                "bqrT": np.broadcast_to(
                    bqr_f, (P, NH * ROPE_DIM)
                ).copy(),
                "Wo": _pack(inp["out_w"][csl, :], P),
                "cosT": cosT,
                "sinT": sinT,
                "tri": tri,
                "eye": eye,
            }
        )

    if _NC is None:
        _NC = build()
    res = run_bass_kernel_spmd(_NC, in_maps, core_ids=list(range(8)))

    corr = (
        inp["value_up_b"].astype(np.float32) @ inp["out_w"].astype(np.float32)
        + inp["out_b"].astype(np.float32)
    )
    out = np.empty((B, S, HIDDEN), np.float32)
    for b in range(B):
        acc = res.results[b * 4]["outT"].astype(np.float32)
        for g in range(1, 4):
            acc += res.results[b * 4 + g]["outT"].astype(np.float32)
        out[b] = acc.T + corr[None, :]
    return out


# revision 68
# speedup vs baseline: 1.0172x; 1.0172x over previous
"""Multi-Head Latent Attention on 8 Trainium2 NeuronCores.

Sharding: core c = (batch b = c//4) x (head-group g = c%4, 4 heads each).

The kv down projection is token-sharded across the 4 cores of a batch
group: core (b, r) computes kv_c^T for token slices {512j + 128r : j} and
the full kv_c is assembled with four pipelined AllGathers (one per
512-token chunk, 0.5 MB each), overlapped with compute.  The query path
needs no collective at all: the host folds W_q = query_down_w @
query_up_w (and the rope analog) so each core computes its 4 heads'
queries directly from x^T — per-core FLOPs for the query side are lower
than sharing the 1536-dim latent, since only 768 output dims are needed.
Each core computes K/V/rope/attention for its 4 heads and a partial
output projection; the host sums the 4 bf16 partials per batch and adds
the output bias (plus the value-up bias folded through out_w, exact
because softmax rows sum to 1).

Phases are kept temporally separate (projections | attention | out-proj)
because the clock governor duty-cycles the PE when many engines run
concurrently; pure-matmul stretches run at a higher p-state.

All layouts are feature-major: x^T, kv_c^T, K^T, Q^T, ctx^T, out^T.
Scores are computed as scores^T[k, q] so probs^T feeds the context
matmul directly.  Softmax denominators are accumulated on the vector
engine (bf16) and reduced across partitions with one ones-matmul per
(chunk, head).  Exp is applied without max-subtraction (scores for this
problem are in [-1, 1]).

Rope is applied token-major: projections computed once with tokens on
partitions (weights streaming), rotated pairwise on the vector engine,
transposed back to feature-major with PE transposes (deferred so the PE
never waits on the rotation chain).
"""

import numpy as np
import ml_dtypes

import concourse.bass as bass
import concourse.mybir as mybir
from concourse.tile import TileContext
from concourse.bass_utils import run_bass_kernel_spmd

F32 = mybir.dt.float32
BF16 = mybir.dt.bfloat16
AF = mybir.ActivationFunctionType
BF = ml_dtypes.bfloat16

HIDDEN = 2048
NUM_HEADS = 16
HEAD_DIM = 128
KV_C = 512
Q_C = 1536
ROPE_DIM = 64
B, S = 2, 2048

P = 128
NH = 4          # heads per core
SC = 512        # token chunk (one AllGather / one attention q-chunk)
NKT = HIDDEN // P       # 16 k-tiles over x features
NKV = KV_C // P         # 4 kv_c feature tiles
NSC = S // SC           # 4 chunks
NT = S // P             # 16 token tiles
SCALE = float(1.0 / np.sqrt(HEAD_DIM + ROPE_DIM))
NEG = -1.0e5
GROUPS = [[0, 1, 2, 3], [4, 5, 6, 7]]


def _split_waits(nc, maxw=1):
    """This container's walrus accepts at most one sem-wait per instruction;
    move excess waits onto same-engine NOPs inserted immediately before."""
    for fn in nc.m.functions:
        for bb in fn.blocks:
            newlist = []
            for ins in bb.instructions:
                si = ins.sync_info
                if si is not None and si.on_wait is not None and len(si.on_wait) > maxw:
                    waits = list(si.on_wait)
                    extra, keep = waits[:-maxw], waits[-maxw:]
                    for k, i in enumerate(range(0, len(extra), maxw)):
                        nop = mybir.InstNoOp(
                            name=f"{ins.name}-waitsplit-{k}", ins=[], outs=[]
                        )
                        nop.engine = ins.engine
                        nop.sync_info = mybir.SyncInfo(
                            on_wait=extra[i : i + maxw], on_update=[]
                        )
                        newlist.append(nop)
                    ins.sync_info = mybir.SyncInfo(
                        on_wait=keep, on_update=list(si.on_update or [])
                    )
                newlist.append(ins)
            bb.instructions = newlist


def build():
    nc = bass.Bass(num_devices=8)
    dt = nc.dram_tensor
    xT = dt("xT", [P, NSC, NKT, SC], BF16, kind="ExternalInput")
    xTo = dt("xTo", [P, NSC, NKT, P], BF16, kind="ExternalInput")
    Wdkv = dt("Wdkv", [P, NKV, NKT, P], BF16, kind="ExternalInput")
    bdkv = dt("bdkv", [P, NKV], F32, kind="ExternalInput")
    Wku = dt("Wku", [P, NKV, NH * HEAD_DIM], BF16, kind="ExternalInput")
    bku = dt("bku", [P, 4], F32, kind="ExternalInput")
    Wvu = dt("Wvu", [P, NKV, NH * HEAD_DIM], BF16, kind="ExternalInput")
    Wkr = dt("Wkr", [P, NKV, NH * ROPE_DIM], BF16, kind="ExternalInput")
    Wq = dt("Wq", [P, 2, NKT, 256], BF16, kind="ExternalInput")
    bq = dt("bq", [P, 4], F32, kind="ExternalInput")
    Wqr = dt("Wqr", [P, NKT, NH * ROPE_DIM], BF16, kind="ExternalInput")
    # rope biases, token-major broadcast: [P, NH*ROPE_DIM] (same per token)
    bkrT = dt("bkrT", [P, NH * ROPE_DIM], F32, kind="ExternalInput")
    bqrT = dt("bqrT", [P, NH * ROPE_DIM], F32, kind="ExternalInput")
    Wo = dt("Wo", [P, NKT, NH, P], BF16, kind="ExternalInput")
    # token-major cos/sin: [P tokens-within-tile, tile, NH, ROPE_DIM/2]
    cosT = dt("cosT", [P, NT * NH * (ROPE_DIM // 2)], BF16, kind="ExternalInput")
    sinT = dt("sinT", [P, NT * NH * (ROPE_DIM // 2)], BF16, kind="ExternalInput")
    tri = dt("tri", [P, P], F32, kind="ExternalInput")
    eye = dt("eye", [P, P], BF16, kind="ExternalInput")
    outT = dt("outT", [HIDDEN, S], BF16, kind="ExternalOutput")

    cc_in = [dt(f"cc_in{j}", [P, NKV, P], BF16, kind="Internal")
             for j in range(NSC)]
    cc_out = [dt(f"cc_out{j}", [4, P, NKV, P], BF16, kind="Internal")
              for j in range(NSC)]

    with TileContext(nc) as tc:
        with (
            tc.tile_pool(name="const", bufs=1) as pc,
            tc.tile_pool(name="qkv", bufs=1) as pq,
        ):
            # --- mid-lived scope: phase 1 + projections (loop 1) ----------
            pmid = tc.alloc_tile_pool(name="pmid", bufs=1)

            # kv-down weights + bias first on gpsimd (phase-1 critical)
            bdkv_sb = pc.tile([P, NKV], F32)
            nc.gpsimd.dma_start(bdkv_sb[:], bdkv[:])
            wdkv_t = pmid.tile([P, NKT, KV_C], BF16, tag="wdkv")
            for q4 in range(4):
                nc.gpsimd.dma_start(
                    wdkv_t[:, :, q4 * P : (q4 + 1) * P], Wdkv[:, q4]
                )
            # remaining constants (gpsimd queue, needed later)
            bq_sb = pc.tile([P, 4], F32)
            nc.gpsimd.dma_start(bq_sb[:], bq[:])
            bku_sb = pc.tile([P, 4], F32)
            nc.gpsimd.dma_start(bku_sb[:], bku[:])
            tri_sb = pc.tile([P, P], F32)
            nc.gpsimd.dma_start(tri_sb[:], tri[:])
            eye_sb = pc.tile([P, P], BF16)
            nc.gpsimd.dma_start(eye_sb[:], eye[:])
            ones_sb = pc.tile([P, P], BF16)
            nc.vector.memset(ones_sb[:], 1.0)
            bkrT_sb = pc.tile([P, NH * ROPE_DIM], F32)
            nc.gpsimd.dma_start(bkrT_sb[:], bkrT[:])
            bqrT_sb = pc.tile([P, NH * ROPE_DIM], F32)
            nc.gpsimd.dma_start(bqrT_sb[:], bqrT[:])

            # persistent across attention: K/V and per-chunk Q/ctx tiles
            kc_sb = pq.tile([P, NH, S], BF16)
            kr_sb = pq.tile([P, 2, S], BF16)
            v_sb = pq.tile([P, NT, NH * HEAD_DIM], BF16)

            cos_sb = pmid.tile([P, NT, NH, ROPE_DIM // 2], BF16, tag="cos")
            nc.gpsimd.dma_start(
                cos_sb[:], cosT.rearrange("p (t h i) -> p t h i", t=NT, h=NH)
            )
            sin_sb = pmid.tile([P, NT, NH, ROPE_DIM // 2], BF16, tag="sin")
            nc.gpsimd.dma_start(
                sin_sb[:], sinT.rearrange("p (t h i) -> p t h i", t=NT, h=NH)
            )
            kv_sb = pmid.tile([P, NKV, S], BF16, tag="kv")

            def load_xt_chunk(qc):
                t = pmid.tile([P, NKT, SC], BF16, tag="xt", bufs=2)
                nc.scalar.dma_start(t[:], xT[:, qc])
                return t

            # remaining weights, spread across the three DMA-capable queues
            # so the phase-1/loop-1 critical loads land early
            wq_t = pmid.tile([P, NKT, NH * HEAD_DIM], BF16, tag="wq")
            wqr_t = pmid.tile([P, NKT, NH * ROPE_DIM], BF16, tag="wqr")
            nc.gpsimd.dma_start(wqr_t[:], Wqr[:])
            wku_t = pmid.tile([P, NKV, NH * HEAD_DIM], BF16, tag="wku")
            wvu_t = pmid.tile([P, NKV, NH * HEAD_DIM], BF16, tag="wvu")
            wkr_t = pmid.tile([P, NKV, NH * ROPE_DIM], BF16, tag="wkr")

            # ---------------- phase 1: kv down projection + AllGather -----
            with (
                tc.tile_pool(name="p1", bufs=1) as p1,
                tc.tile_pool(name="ps1", bufs=6, space="PSUM") as ps1,
            ):
                # own token columns for the kv down projection (host-packed),
                # split across sync+scalar queues to land fast
                xto = p1.tile([P, NKT, NSC * P], BF16)
                for j in range(NSC):
                    eng = nc.sync if j % 2 == 0 else nc.scalar
                    eng.dma_start(
                        xto[:, :, j * P : (j + 1) * P], xTo[:, j]
                    )
                # first x chunk + query weights on scalar, behind the
                # critical xto halves; sync stays clear for cc_in posts
                xt0 = load_xt_chunk(0)
                for h2 in range(2):
                    nc.scalar.dma_start(
                        wq_t[:, :, h2 * 256 : (h2 + 1) * 256], Wq[:, h2]
                    )
                nc.scalar.dma_start(wku_t[:], Wku[:])
                nc.scalar.dma_start(wvu_t[:], Wvu[:])
                nc.scalar.dma_start(wkr_t[:], Wkr[:])
                kv_own = p1.tile([P, NKV, NSC * P], BF16)
                for j in range(NSC):
                    for m in range(NKV):
                        ps = ps1.tile([P, P], F32, tag="mm")
                        for k in range(NKT):
                            nc.tensor.matmul(
                                ps[:],
                                wdkv_t[:, k, m * P : (m + 1) * P],
                                xto[:, k, j * P : (j + 1) * P],
                                start=(k == 0),
                                stop=(k == NKT - 1),
                            )
                        nc.vector.tensor_scalar_add(
                            kv_own[:, m, j * P : (j + 1) * P],
                            ps[:],
                            bdkv_sb[:, m : m + 1],
                        )
                    nc.sync.dma_start(
                        cc_in[j][:], kv_own[:, :, j * P : (j + 1) * P]
                    )
                for j in range(NSC):
                    nc.gpsimd.collective_compute(
                        "AllGather",
                        mybir.AluOpType.bypass,
                        replica_groups=GROUPS,
                        ins=[cc_in[j][:].opt()],
                        outs=[cc_out[j][:].opt()],
                    )
                for j in range(NSC):
                    for rr in range(4):
                        c0 = j * SC + rr * P
                        nc.sync.dma_start(
                            kv_sb[:, :, c0 : c0 + P], cc_out[j][rr]
                        )

            # --------- loop 1: all projections (PE-dense) -----------------
            q_tiles = []
            with (
                tc.tile_pool(name="p2t", bufs=3) as p2t,
                tc.tile_pool(name="psP", bufs=1, space="PSUM") as psP,
            ):

                def rope_project(tt, nk, src, src_t, w_t, bias_sb):
                    """Project rope dims token-major for global tile tt and
                    rotate on DVE; returns the rotated bf16 tile."""
                    ps = psP.tile([P, NH * ROPE_DIM], F32, tag="mm", bufs=4)
                    for k in range(nk):
                        nc.tensor.matmul(
                            ps[:],
                            src[:, k, src_t * P : (src_t + 1) * P],
                            w_t[:, k, :],
                            start=(k == 0),
                            stop=(k == nk - 1),
                        )
                    pre = p2t.tile([P, NH * ROPE_DIM], BF16, tag="ropeadd")
                    nc.vector.tensor_tensor(
                        pre[:], ps[:], bias_sb[:], mybir.AluOpType.add
                    )
                    prr = pre.rearrange("p (h i two) -> p h i two", h=NH, two=2)
                    rot = p2t.tile([P, NH, ROPE_DIM // 2, 2], BF16, tag="rot",
                                   bufs=5)
                    t1 = p2t.tile([P, NH, ROPE_DIM // 2], BF16, tag="ropet1")
                    nc.vector.tensor_tensor(
                        t1[:], prr[:, :, :, 0], cos_sb[:, tt],
                        mybir.AluOpType.mult,
                    )
                    t2 = p2t.tile([P, NH, ROPE_DIM // 2], BF16, tag="ropet2")
                    nc.vector.tensor_tensor(
                        t2[:], prr[:, :, :, 1], sin_sb[:, tt],
                        mybir.AluOpType.mult,
                    )
                    nc.vector.tensor_tensor(
                        rot[:, :, :, 0], t1[:], t2[:], mybir.AluOpType.subtract
                    )
                    nc.vector.tensor_tensor(
                        t1[:], prr[:, :, :, 0], sin_sb[:, tt],
                        mybir.AluOpType.mult,
                    )
                    nc.vector.tensor_tensor(
                        t2[:], prr[:, :, :, 1], cos_sb[:, tt],
                        mybir.AluOpType.mult,
                    )
                    nc.vector.tensor_tensor(
                        rot[:, :, :, 1], t1[:], t2[:], mybir.AluOpType.add
                    )
                    return rot

                def rope_store(rot, dst_sb, dst_t):
                    """Transpose the rotated tile back to feature-major."""
                    rotf = rot.rearrange("p h i two -> p (h i two)")
                    for hb in range(2):
                        pt = psP.tile([P, P], BF16, tag="tp", bufs=4)
                        nc.tensor.transpose(
                            pt[:], rotf[:, hb * P : (hb + 1) * P], eye_sb[:]
                        )
                        nc.scalar.copy(
                            dst_sb[:, hb, dst_t * P : (dst_t + 1) * P], pt[:]
                        )

                xt_cur = xt0
                for qc in range(NSC):
                    qsl = slice(qc * SC, (qc + 1) * SC)
                    xt = xt_cur
                    if qc + 1 < NSC:
                        xt_cur = load_xt_chunk(qc + 1)
                    # queries for this chunk: direct from x, no AG dep
                    qc_sb = pq.tile([P, NH, SC], BF16, tag="qc", bufs=4)
                    qr_sb = pq.tile([P, 2, SC], BF16, tag="qr", bufs=4)
                    q_tiles.append((qc_sb, qr_sb))
                    for m in range(NH):
                        ps = psP.tile([P, SC], F32, tag="mm", bufs=4)
                        for k in range(NKT):
                            nc.tensor.matmul(
                                ps[:],
                                wq_t[:, k, m * P : (m + 1) * P],
                                xt[:, k, :],
                                start=(k == 0),
                                stop=(k == NKT - 1),
                            )
                        nc.vector.tensor_scalar_add(
                            qc_sb[:, m, :], ps[:], bq_sb[:, m : m + 1]
                        )
                    rots = [
                        rope_project(qc * 4 + t, NKT, xt, t, wqr_t, bqrT_sb)
                        for t in range(4)
                    ]
                    for t in range(4):
                        rope_store(rots[t], qr_sb, t)

                    # keys/values for this chunk (needs AllGather qc)
                    for m in range(NH):
                        ps = psP.tile([P, SC], F32, tag="mm", bufs=4)
                        for k in range(NKV):
                            nc.tensor.matmul(
                                ps[:],
                                wku_t[:, k, m * P : (m + 1) * P],
                                kv_sb[:, k, qsl],
                                start=(k == 0),
                                stop=(k == NKV - 1),
                            )
                        nc.vector.tensor_scalar_add(
                            kc_sb[:, m, qsl], ps[:], bku_sb[:, m : m + 1]
                        )
                    for t in range(4):
                        tt = qc * 4 + t
                        ps = psP.tile([P, NH * HEAD_DIM], F32, tag="mm",
                                      bufs=4)
                        for k in range(NKV):
                            nc.tensor.matmul(
                                ps[:],
                                kv_sb[:, k, tt * P : (tt + 1) * P],
                                wvu_t[:, k, :],
                                start=(k == 0),
                                stop=(k == NKV - 1),
                            )
                        nc.vector.tensor_copy(v_sb[:, tt, :], ps[:])
                    rots = [
                        rope_project(qc * 4 + t, NKV, kv_sb, qc * 4 + t,
                                     wkr_t, bkrT_sb)
                        for t in range(4)
                    ]
                    for t in range(4):
                        rope_store(rots[t], kr_sb, qc * 4 + t)
            pmid.release()

            # --------- loop 2: attention --------------------------------
            ctx_tiles = []
            patx = tc.alloc_tile_pool(name="patx", bufs=1)
            with (
                tc.tile_pool(name="pat", bufs=8) as pat,
                tc.tile_pool(name="patt", bufs=2) as patt,
                tc.tile_pool(name="psL2", bufs=1, space="PSUM") as psL2,
            ):
                for qc in range(NSC):
                    qc_sb, qr_sb = q_tiles[qc]
                    nkb = 4 * qc + 4
                    ctx_q = patx.tile([P, NH, SC], BF16, tag="ctx", bufs=4)
                    ctx_tiles.append(ctx_q)
                    for h in range(NH):
                        hc = h // 2
                        hp = (h % 2) * ROPE_DIM
                        psum_ctx = psL2.tile([P, SC], F32, tag="ctx", bufs=2)
                        acc = patt.tile([P, SC], BF16, tag="acc")

                        def score_mm(dst, kb, c):
                            nc.tensor.matmul(
                                dst[:, c:],
                                kc_sb[:, h, kb * P : (kb + 1) * P],
                                qc_sb[:, h, c:],
                                start=True, stop=False,
                            )
                            nc.tensor.matmul(
                                dst[:, c:],
                                kr_sb[hp : hp + ROPE_DIM, hc,
                                      kb * P : (kb + 1) * P],
                                qr_sb[hp : hp + ROPE_DIM, hc, c:],
                                start=False, stop=True,
                            )

                        def acc_ctx(probs, kb, c):
                            if kb == 0:
                                nc.vector.tensor_copy(acc[:], probs[:])
                            else:
                                nc.vector.tensor_tensor(
                                    acc[:, c:], acc[:, c:], probs[:, c:],
                                    mybir.AluOpType.add,
                                )
                            nc.tensor.matmul(
                                psum_ctx[:, c:],
                                v_sb[:, kb, h * P : (h + 1) * P],
                                probs[:, c:],
                                start=(kb == 0), stop=(kb == nkb - 1),
                            )

                        # Emit score-matmuls one step ahead of the ctx
                        # matmuls so the PE never waits on the exp latency.
                        nfull = 4 * qc
                        steps = [("pair", kb) for kb in range(0, nfull, 2)]
                        steps += [("diag", kb) for kb in range(nfull, nkb)]

                        def emit_scores(step):
                            kind, kb = step
                            if kind == "pair":
                                ps = psL2.tile([P, 2, SC], F32, tag="sc2",
                                               bufs=2)
                                score_mm(ps[:, 0], kb, 0)
                                score_mm(ps[:, 1], kb + 1, 0)
                                probs2 = pat.tile([P, 2, SC], BF16,
                                                  tag="probs2", bufs=4)
                                nc.scalar.activation(
                                    probs2[:], ps[:], AF.Exp, scale=SCALE
                                )
                                return probs2
                            ps = psL2.tile([P, SC], F32, tag="sc", bufs=2)
                            c = (kb - 4 * qc) * P
                            score_mm(ps, kb, c)
                            nc.vector.tensor_tensor(
                                ps[:, c : c + P],
                                ps[:, c : c + P],
                                tri_sb[:],
                                mybir.AluOpType.add,
                            )
                            probs = pat.tile([P, SC], BF16, tag="probs",
                                             bufs=4)
                            nc.scalar.activation(
                                probs[:, c:], ps[:, c:], AF.Exp, scale=SCALE
                            )
                            return probs

                        def emit_ctx(step, probs):
                            kind, kb = step
                            if kind == "pair":
                                acc_ctx(probs[:, 0], kb, 0)
                                acc_ctx(probs[:, 1], kb + 1, 0)
                            else:
                                acc_ctx(probs, kb, (kb - 4 * qc) * P)

                        prev = None
                        for step in steps:
                            probs = emit_scores(step)
                            if prev is not None:
                                emit_ctx(*prev)
                            prev = (step, probs)
                        emit_ctx(*prev)
                        psd = psL2.tile([P, SC], F32, tag="sc", bufs=2)
                        nc.tensor.matmul(
                            psd[:], ones_sb[:], acc[:], start=True, stop=True
                        )
                        rcp = patt.tile([P, SC], F32, tag="rcp", bufs=2)
                        nc.vector.reciprocal(rcp[:], psd[:])
                        nc.vector.tensor_tensor(
                            ctx_q[:, h, :], psum_ctx[:], rcp[:],
                            mybir.AluOpType.mult,
                        )

            # --------- loop 3: out-projection (pure matmul) ---------------
            with (
                tc.tile_pool(name="pout", bufs=6) as pout,
                tc.tile_pool(name="pow", bufs=3) as pow_,
                tc.tile_pool(name="psL3", bufs=1, space="PSUM") as psL3,
            ):
                for m in range(NKT):
                    wo_t = pow_.tile([P, NH, P], BF16, tag="wo")
                    nc.gpsimd.dma_start(wo_t[:], Wo[:, m])
                    for qc in range(NSC):
                        ps = psL3.tile([P, SC], F32, tag="m", bufs=4)
                        for k in range(NH):
                            nc.tensor.matmul(
                                ps[:],
                                wo_t[:, k, :],
                                ctx_tiles[qc][:, k, :],
                                start=(k == 0),
                                stop=(k == NH - 1),
                            )
                        og = pout.tile([P, SC], BF16, tag="og")
                        nc.vector.tensor_copy(og[:], ps[:])
                        nc.sync.dma_start(
                            outT[m * P : (m + 1) * P,
                                 qc * SC : (qc + 1) * SC],
                            og[:],
                        )
            patx.release()
    _split_waits(nc)
    return nc


def _col_bias(b, nm):
    """[nm*128] -> [128, nm] (column m = bias for feature chunk m)."""
    return np.ascontiguousarray(b.reshape(nm, P).T).astype(np.float32)


def _pack(w, msplit=None):
    """[K, M] -> [P, (M//msplit), K//P, msplit] if msplit else [P, K//P, M]:
    partition-major device layout so DMAs are contiguous per partition."""
    K, M = w.shape
    a = w.reshape(K // P, P, M)
    if msplit is None:
        return np.ascontiguousarray(a.transpose(1, 0, 2)).astype(BF)
    a = a.reshape(K // P, P, M // msplit, msplit)
    return np.ascontiguousarray(a.transpose(1, 2, 0, 3)).astype(BF)


_NC = None


def kernel(**inputs):
    global _NC
    inp = {k: np.asarray(v) for k, v in inputs.items()}
    x = inp["x"].astype(np.float32)

    # token-major cos/sin: cosT[p, (t, h, i)] = cos(pos(t*128+p)*inv_freq[i])
    pos = np.arange(S, dtype=np.float64)
    inv = 1.0 / (10000.0 ** (np.arange(0, ROPE_DIM, 2, np.float64) / ROPE_DIM))
    ang = pos[:, None] * inv[None, :]              # [S, 32]
    cosS = np.cos(ang).reshape(NT, P, ROPE_DIM // 2)  # [t, p, i]
    sinS = np.sin(ang).reshape(NT, P, ROPE_DIM // 2)
    cosT = np.repeat(
        cosS.transpose(1, 0, 2)[:, :, None, :], NH, axis=2
    ).reshape(P, -1).astype(BF)
    sinT = np.repeat(
        sinS.transpose(1, 0, 2)[:, :, None, :], NH, axis=2
    ).reshape(P, -1).astype(BF)
    tri = np.where(
        np.arange(P)[None, :] >= np.arange(P)[:, None], 0.0, NEG
    ).astype(np.float32)
    eye = np.eye(P, dtype=np.float32).astype(BF)

    qdw = inp["query_down_w"].astype(np.float32)
    qdb = inp["query_down_b"].astype(np.float32)

    in_maps = []
    for c in range(8):
        b, r = c // 4, c % 4
        h0 = r * NH
        csl = slice(h0 * HEAD_DIM, (h0 + NH) * HEAD_DIM)
        rsl = slice(h0 * ROPE_DIM, (h0 + NH) * ROPE_DIM)
        # fold the query path: Q = x @ (qdw @ qu) + (qdb @ qu + qub)
        wq = qdw @ inp["query_up_w"][:, csl].astype(np.float32)
        bq_f = qdb @ inp["query_up_w"][:, csl].astype(np.float32) \
            + inp["query_up_b"][csl].astype(np.float32)
        wqr = qdw @ inp["query_rope_w"][:, rsl].astype(np.float32)
        bqr_f = qdb @ inp["query_rope_w"][:, rsl].astype(np.float32) \
            + inp["query_rope_b"][rsl].astype(np.float32)
        own_cols = np.concatenate(
            [np.arange(SC * j + P * r, SC * j + P * r + P) for j in range(NSC)]
        )
        in_maps.append(
            {
                "xT": _pack(x[b].T, SC),
                "xTo": _pack(x[b].T[:, own_cols], P),
                "Wdkv": _pack(inp["kv_down_w"], P),
                "bdkv": _col_bias(inp["kv_down_b"], NKV),
                "Wku": _pack(inp["key_up_w"][:, csl]),
                "bku": _col_bias(inp["key_up_b"][csl], 4),
                "Wvu": _pack(inp["value_up_w"][:, csl]),
                "Wkr": _pack(inp["key_rope_w"][:, rsl]),
                "Wq": _pack(wq, 256),
                "bq": _col_bias(bq_f, 4),
                "Wqr": _pack(wqr),
                "bkrT": np.broadcast_to(
                    inp["key_rope_b"][rsl].astype(np.float32),
                    (P, NH * ROPE_DIM),
                ).copy(),
                "bqrT": np.broadcast_to(
                    bqr_f, (P, NH * ROPE_DIM)
                ).copy(),
                "Wo": _pack(inp["out_w"][csl, :], P),
                "cosT": cosT,
                "sinT": sinT,
                "tri": tri,
                "eye": eye,
            }
        )

    if _NC is None:
        _NC = build()
    res = run_bass_kernel_spmd(_NC, in_maps, core_ids=list(range(8)))

    corr = (
        inp["value_up_b"].astype(np.float32) @ inp["out_w"].astype(np.float32)
        + inp["out_b"].astype(np.float32)
    )
    out = np.empty((B, S, HIDDEN), np.float32)
    for b in range(B):
        acc = res.results[b * 4]["outT"].astype(np.float32)
        for g in range(1, 4):
            acc += res.results[b * 4 + g]["outT"].astype(np.float32)
        out[b] = acc.T + corr[None, :]
    return out
